# revision 16
# baseline (speedup 1.0000x reference)
"""LIF spiking-neuron kernel for Trainium2 (Bass/Tile), 8-core data-parallel.

Reference semantics (per element, scan over T=8):
    mem = mem * 0.5 + x_t
    s_t = (mem > 1.0) ? 1.0 : 0.0        # forward value of the spike
    mem = mem - s_t

The kernel carries neg_mem = -mem so each step is two fused
scalar_tensor_tensor ops on DVE; the spike output is produced on the
otherwise-idle ACT engine as Relu(Sign(m - 1)) (exact: m-1 is Sterbenz-exact
near the threshold, Sign(0)=0 preserves the strict >):
    m        = (neg_mem * -0.5) + x_t    # DVE stt: mult, add
    sg       = Sign(m - 1)               # ACT activation(Sign, bias=-1)
    s_t      = Relu(sg)                  # ACT activation(Relu)  (output tile)
    neg_mem' = (m > 1.0) - m             # DVE stt: is_gt, subtract == -(m-s)

GPSIMD is deliberately unused: tensor_scalar is_gt on it measured ~8.5
G elem/s (499us total) vs DVE's 123 G elem/s. Bacc.generate_event_semaphores
splits multi-waits, so the 1-sync-wait limit of the custom STT opcode is
satisfied by construction.

Sharding: batch dim B=32 (dim 1 after temporal expand) split across 8 cores,
4 per core. Per-core tensor is [T=8, 128 partitions, 4096 free] fp32; the
free axis is tiled into chunks, each chunk runs the 8-step scan with its
membrane state resident in SBUF.
"""

import numpy as np

import concourse.bass as bass
import concourse.bacc as bacc
import concourse.tile as tile
from concourse import mybir
from concourse.bass_utils import run_bass_kernel_spmd

T = 8
B = 32
C = 128
H = 32
W = 32
NCORES = 8
BL = B // NCORES              # 4 batch elements per core
N = BL * C * H * W            # 524288 elements per timestep per core
P = 128                       # SBUF partitions
FREE = N // P                 # 4096 fp32 per partition per timestep
FCHUNK = 2048                 # free-dim chunk size

_ALU = mybir.AluOpType


def build_bass(fchunk: int = FCHUNK, free: int = FREE):
    nc = bacc.Bacc("TRN2", target_bir_lowering=False, debug=False,
                   num_devices=NCORES)
    x_ap = nc.dram_tensor("x", [T, P, free], mybir.dt.float32,
                          kind="ExternalInput").ap()
    o_ap = nc.dram_tensor("out", [T, P, free], mybir.dt.float32,
                          kind="ExternalOutput").ap()

    nchunks = free // fchunk
    with tile.TileContext(nc) as tc:
        with (
            tc.tile_pool(name="xp", bufs=5) as xp,
            tc.tile_pool(name="sp", bufs=4) as sp,
            tc.tile_pool(name="sgp", bufs=2) as sgp,
            tc.tile_pool(name="mp", bufs=3) as mp,
            tc.tile_pool(name="cp", bufs=1) as cp,
        ):
            _F = mybir.ActivationFunctionType
            neg1 = cp.tile([P, 1], mybir.dt.float32, tag="neg1")
            nc.gpsimd.memset(neg1[:], -1.0)
            # Steps whose membrane update runs on the plain path
            # (ACT spike + GPSIMD sub) instead of the fused DVE stt2;
            # balances DVE / ACT / GPSIMD occupancy.
            PLAIN = (1, 3, 5)
            for ci in range(nchunks):
                sl = bass.ts(ci, fchunk)
                state = None          # membrane tile carried across steps
                state_neg = True      # True: state == -mem, else state == mem
                for t in range(T):
                    xt = xp.tile([P, fchunk], mybir.dt.float32, tag="x")
                    nc.sync.dma_start(xt[:], x_ap[t, :, sl])
                    if t == 0:
                        m = xt
                    else:
                        m = mp.tile([P, fchunk], mybir.dt.float32, tag="m")
                        nc.vector.scalar_tensor_tensor(
                            m[:], state[:], -0.5 if state_neg else 0.5, xt[:],
                            _ALU.mult, _ALU.add)
                    if t == T - 1 or t in PLAIN:
                        # spike on ACT: s = Relu(Sign(m - 1)), exact {0,1}
                        sg = sgp.tile([P, fchunk], mybir.dt.float32, tag="sg")
                        nc.scalar.activation(sg[:], m[:], _F.Sign,
                                             bias=neg1[:])
                        s = sp.tile([P, fchunk], mybir.dt.float32, tag="s")
                        nc.scalar.activation(s[:], sg[:], _F.Relu)
                        if t < T - 1:
                            state = mp.tile([P, fchunk], mybir.dt.float32,
                                            tag="pm")
                            nc.gpsimd.tensor_sub(state[:], m[:], s[:])
                            state_neg = False
                    else:
                        # fused update on DVE: state = (m>1) - m == -mem'
                        state = mp.tile([P, fchunk], mybir.dt.float32,
                                        tag="nm")
                        nc.vector.scalar_tensor_tensor(
                            state[:], m[:], 1.0, m[:],
                            _ALU.is_gt, _ALU.subtract)
                        state_neg = True
                        # spike on GPSIMD: s = m + (-mem') == (m>1), exact
                        s = sp.tile([P, fchunk], mybir.dt.float32, tag="s")
                        nc.gpsimd.tensor_add(s[:], m[:], state[:])
                    # Stores go out on the second HWDGE queue (scalar) so
                    # loads (sync queue) and stores run on parallel rings.
                    nc.scalar.dma_start(o_ap[t, :, sl], s[:])
    nc.compile()
    return nc


_NC_CACHE: dict = {}


def _get_nc():
    if "nc" not in _NC_CACHE:
        _NC_CACHE["nc"] = build_bass()
    return _NC_CACHE["nc"]


def kernel(x: np.ndarray) -> np.ndarray:
    x = np.asarray(x)
    assert x.shape == (T * B, C, H, W), x.shape
    in_dtype = x.dtype
    xs = x.reshape(T, B, C, H, W)

    in_maps = []
    for i in range(NCORES):
        xi = np.ascontiguousarray(xs[:, i * BL:(i + 1) * BL])
        in_maps.append({"x": xi.reshape(T, P, FREE)})

    nc = _get_nc()
    res = run_bass_kernel_spmd(nc, in_maps, list(range(NCORES)))

    out = np.empty((T, B, C, H, W), dtype=np.float32)
    for i in range(NCORES):
        out[:, i * BL:(i + 1) * BL] = res.results[i]["out"].reshape(
            T, BL, C, H, W)
    return out.reshape(T * B, C, H, W).astype(in_dtype, copy=False)


# revision 19
# speedup vs baseline: 1.6489x; 1.6489x over previous
"""LIF spiking-neuron kernel for Trainium2 (Bass/Tile), 8-core data-parallel.

Reference semantics (per element, scan over T=8):
    mem = mem * 0.5 + x_t
    s_t = (mem > 1.0) ? 1.0 : 0.0        # forward value of the spike
    mem = mem - s_t

The kernel carries neg_mem = -mem so each step is two fused
scalar_tensor_tensor ops on DVE; the spike output is produced on the
otherwise-idle ACT engine as Relu(Sign(m - 1)) (exact: m-1 is Sterbenz-exact
near the threshold, Sign(0)=0 preserves the strict >):
    m        = (neg_mem * -0.5) + x_t    # DVE stt: mult, add
    sg       = Sign(m - 1)               # ACT activation(Sign, bias=-1)
    s_t      = Relu(sg)                  # ACT activation(Relu)  (output tile)
    neg_mem' = (m > 1.0) - m             # DVE stt: is_gt, subtract == -(m-s)

GPSIMD is deliberately unused: tensor_scalar is_gt on it measured ~8.5
G elem/s (499us total) vs DVE's 123 G elem/s. Bacc.generate_event_semaphores
splits multi-waits, so the 1-sync-wait limit of the custom STT opcode is
satisfied by construction.

Sharding: batch dim B=32 (dim 1 after temporal expand) split across 8 cores,
4 per core. Per-core tensor is [T=8, 128 partitions, 4096 free] fp32; the
free axis is tiled into chunks, each chunk runs the 8-step scan with its
membrane state resident in SBUF.
"""

import numpy as np

import concourse.bass as bass
import concourse.bacc as bacc
import concourse.tile as tile
from concourse import mybir
from concourse.bass_utils import run_bass_kernel_spmd

T = 8
B = 32
C = 128
H = 32
W = 32
NCORES = 8
BL = B // NCORES              # 4 batch elements per core
N = BL * C * H * W            # 524288 elements per timestep per core
P = 128                       # SBUF partitions
FREE = N // P                 # 4096 fp32 per partition per timestep
FCHUNK = 4096                 # free-dim chunk size

_ALU = mybir.AluOpType


def build_bass(fchunk: int = FCHUNK, free: int = FREE):
    nc = bacc.Bacc("TRN2", target_bir_lowering=False, debug=False,
                   num_devices=NCORES)
    x_ap = nc.dram_tensor("x", [T, P, free], mybir.dt.float32,
                          kind="ExternalInput").ap()
    o_ap = nc.dram_tensor("out", [T, P, free], mybir.dt.float32,
                          kind="ExternalOutput").ap()

    nchunks = free // fchunk
    with tile.TileContext(nc) as tc:
        with (
            tc.tile_pool(name="xp", bufs=3) as xp,
            tc.tile_pool(name="sp", bufs=3) as sp,
            tc.tile_pool(name="mp", bufs=2) as mp,
            tc.tile_pool(name="cp", bufs=1) as cp,
        ):
            _F = mybir.ActivationFunctionType
            neg1 = cp.tile([P, 1], mybir.dt.float32, tag="neg1")
            nc.gpsimd.memset(neg1[:], -1.0)
            for ci in range(nchunks):
                sl = bass.ts(ci, fchunk)
                neg_mem = None
                for t in range(T):
                    xt = xp.tile([P, fchunk], mybir.dt.float32, tag="x")
                    nc.sync.dma_start(xt[:], x_ap[t, :, sl])
                    if t == 0:
                        m = xt
                    else:
                        m = mp.tile([P, fchunk], mybir.dt.float32, tag="m")
                        nc.vector.scalar_tensor_tensor(
                            m[:], neg_mem[:], -0.5, xt[:],
                            _ALU.mult, _ALU.add)
                    # spike on ACT, in-place: s = Relu(Sign(m-1)), exact {0,1}
                    s = sp.tile([P, fchunk], mybir.dt.float32, tag="s")
                    nc.scalar.activation(s[:], m[:], _F.Sign, bias=neg1[:])
                    nc.scalar.activation(s[:], s[:], _F.Relu)
                    if t < T - 1:
                        # fused update on DVE: neg_mem' = (m>1) - m
                        neg_mem = mp.tile([P, fchunk], mybir.dt.float32,
                                          tag="nm")
                        nc.vector.scalar_tensor_tensor(
                            neg_mem[:], m[:], 1.0, m[:],
                            _ALU.is_gt, _ALU.subtract)
                    # Stores go out on the second HWDGE queue (scalar) so
                    # loads (sync queue) and stores run on parallel rings.
                    nc.scalar.dma_start(o_ap[t, :, sl], s[:])
    nc.compile()
    return nc


_NC_CACHE: dict = {}


def _get_nc():
    if "nc" not in _NC_CACHE:
        _NC_CACHE["nc"] = build_bass()
    return _NC_CACHE["nc"]


def kernel(x: np.ndarray) -> np.ndarray:
    x = np.asarray(x)
    assert x.shape == (T * B, C, H, W), x.shape
    in_dtype = x.dtype
    xs = x.reshape(T, B, C, H, W)

    in_maps = []
    for i in range(NCORES):
        xi = np.ascontiguousarray(xs[:, i * BL:(i + 1) * BL])
        in_maps.append({"x": xi.reshape(T, P, FREE)})

    nc = _get_nc()
    res = run_bass_kernel_spmd(nc, in_maps, list(range(NCORES)))

    out = np.empty((T, B, C, H, W), dtype=np.float32)
    for i in range(NCORES):
        out[:, i * BL:(i + 1) * BL] = res.results[i]["out"].reshape(
            T, BL, C, H, W)
    return out.reshape(T * B, C, H, W).astype(in_dtype, copy=False)
